# revision 26
# baseline (speedup 1.0000x reference)
"""YOLO-style loss kernel for Trainium2, SPMD over 8 NeuronCores.

Inputs (full): pred_tensor [32768,7,7,30] f32, target_tensor [32768,7,7,30] f32.
Output: np.ndarray shape (5,) f32 = (loss_xy, loss_wh, loss_obj, loss_noobj, loss_class).

Strategy: pure data parallel on batch dim; each core gets 4096 samples
(200704 cells). Host converts to fp16 and splits channels into fully
contiguous groups so the hot DVE ops coalesce into the 2x packed mode
(strided views of an interleaved [n,10] tile measure 1x or worse):
  - xy4  [n,4] cell-major (x0,y0,x1,y1)        both tensors
  - wh4  [n,4] cell-major (w0,h0,w1,h1)        both tensors
  - cf2  [n,2] cell-major (c0,c1)              both tensors
  - cls  [20,n] channel-major per chunk        both tensors
Per 392-cell chunk: IoU responsibility + five masked squared-diff partial
sums, fused on-chip. Weighted reductions run as premask-multiply (masks are
exactly 0/1) + in-place Square with accum_out on the scalar engine; the
class premask ANDs int32-reinterpreted fp16 pairs against a 0xFFFF mask;
reciprocal via the ~1cpe approx custom-DVE op. Each core returns a [128,20]
f32 partial-sum tile (5 losses x 4 chunks); host reduces and divides by N.
"""

import os
import sys

sys.path.insert(0, "/opt/trn_rl_repo")

import numpy as np

import concourse.bass as bass
import concourse.bacc as bacc
import concourse.tile as tile
from concourse import mybir
from concourse import bass_utils

F32 = mybir.dt.float32
F16 = mybir.dt.float16
I16 = mybir.dt.int16
I32 = mybir.dt.int32
ALU = mybir.AluOpType
ACT = mybir.ActivationFunctionType

S = 7
B = 2
C = 20
D = 30
N_FULL = 32768
N_CORES = 8
N_SHARD = N_FULL // N_CORES            # 4096 samples per core
R = N_SHARD * S * S                    # 200704 cells per core
P = 128                                # partitions
RP = R // P                            # 1568 cells per partition
NCK = 392                              # cells per partition per chunk
N_CH = RP // NCK                       # 4 chunks

PERM_XY = [0, 1, 5, 6]   # x0,y0,x1,y1
PERM_WH = [2, 3, 7, 8]   # w0,h0,w1,h1
PERM_CF = [4, 9]         # c0,c1


def _mk(ap, dims):
    """Rebuild the free dims of `ap` (keeping partition dim + offset) as
    `dims` = list of (step, count)."""
    new = [list(ap.ap[0])] + [[s, c] for s, c in dims]
    return bass.AP(tensor=ap.tensor, offset=ap.offset, ap=new)


def _ins(ap, pos, step, count):
    new = [list(x) for x in ap.ap]
    new.insert(pos, [step, count])
    return bass.AP(tensor=ap.tensor, offset=ap.offset, ap=new)


def build_program():
    nc = bacc.Bacc("TRN2", target_bir_lowering=False, debug=False)
    n = NCK

    def din(name, per_chunk):
        return nc.dram_tensor(name, [P, N_CH * per_chunk], F16, kind="ExternalInput")

    pbox, tbox = din("pbox", n * 10), din("tbox", n * 10)
    pcl, tcl = din("pcl", C * n), din("tcl", C * n)
    out = nc.dram_tensor("out", [P, 5 * N_CH], F32, kind="ExternalOutput")

    pbox_v = pbox.ap().rearrange("p (k a) -> p k a", k=N_CH, a=n * 10)
    tbox_v = tbox.ap().rearrange("p (k a) -> p k a", k=N_CH, a=n * 10)
    pcl_v = pcl.ap().rearrange("p (k c i) -> p k c i", k=N_CH, c=C, i=n)
    tcl_v = tcl.ap().rearrange("p (k c i) -> p k c i", k=N_CH, c=C, i=n)

    with tile.TileContext(nc) as tc:
        with (
            tc.tile_pool(name="raw", bufs=2) as raw,
            tc.tile_pool(name="tmp", bufs=1) as tmp,
            tc.tile_pool(name="persist", bufs=1) as persist,
        ):
            acc = persist.tile([P, 5 * N_CH], F32)

            for k in range(N_CH):
                # one block-major box DMA per tensor: [xy4(4n) | wh4(4n) | cf2(2n)]
                # contiguous inside the transfer, so every group view coalesces
                Bp = raw.tile([P, 10 * n], F16, tag="Bp")
                Bt = raw.tile([P, 10 * n], F16, tag="Bt")
                Pcl = raw.tile([P, C, n], F16, tag="Pcl")
                Tcl = raw.tile([P, C, n], F16, tag="Tcl")
                # box DMAs first; the cls DMA issues are deferred to just
                # before class_block so chunk-0 compute starts after only two
                # serial descriptor issues on the sync engine (head -2us);
                # prefetched chunks still issue a full chunk ahead
                nc.sync.dma_start(out=Bp, in_=pbox_v[:, k])
                nc.sync.dma_start(out=Bt, in_=tbox_v[:, k])

                # box-major rows: x0,y0,x1,y1 | w0,h0,w1,h1 | c0,c1, each a
                # contiguous n-row, so every group op fully coalesces
                Pxy = Bp[:, 0:4 * n]
                Pwh = Bp[:, 4 * n:8 * n]
                Pcf = Bp[:, 8 * n:10 * n]
                Txy = Bt[:, 0:4 * n]
                Twh = Bt[:, 4 * n:8 * n]
                Tcf = Bt[:, 8 * n:10 * n]
                obj_src = Bt[:, 8 * n:9 * n]    # target c0 row, compact [P,n]

                def sqacc(dm, col):
                    # in-place square: ACT streams read-then-write per element,
                    # so out == in is safe and avoids junk tiles whose reuse
                    # would couple engines across chunks
                    nc.scalar.activation(
                        dm, dm, ACT.Square,
                        accum_out=acc[:, 5 * k + col:5 * k + col + 1],
                    )

                def class_block():
                    # class (channel-major [P,20,n]): AND the fp16 diffs
                    # against a 0xFFFF/0x0000 obj mask through int32 views —
                    # pair-packing halves the cost vs a 1x broadcast multiply
                    ffi = tmp.tile([P, n], I16, tag="ffi")
                    nc.scalar.activation(ffi, obj_src, ACT.Copy, scale=-1.0)
                    ff32 = ffi.bitcast(I32)                       # [P, n/2]
                    ff32b = _mk(ff32[:, 0], [(0, C), (1, n // 2)])
                    dcl = tmp.tile([P, C, n], F16, tag="dcl")
                    dmcl = tmp.tile([P, C, n], F16, tag="dmcl")
                    nc.vector.tensor_tensor(dcl, Tcl, Pcl, op=ALU.subtract)
                    nc.vector.tensor_tensor(
                        dmcl.bitcast(I32), dcl.bitcast(I32), ff32b,
                        op=ALU.bitwise_and,
                    )
                    sqacc(dmcl, 4)

                # ---- IoU stage (coords scaled x7: corners 3.5*wh -+ xy) ----
                # single-input scale/clamp ops ride the scalar engine (slack)
                t1 = tmp.tile([P, 4, n], F16, tag="t1")
                nc.scalar.activation(t1, Pwh, ACT.Copy, scale=3.5)
                nl4 = tmp.tile([P, 4, n], F16, tag="nl4")    # -(7l) both boxes
                r4 = tmp.tile([P, 4, n], F16, tag="r4")      # 7r both boxes
                nc.vector.tensor_tensor(nl4, t1, Pxy, op=ALU.subtract)
                nc.vector.tensor_tensor(r4, t1, Pxy, op=ALU.add)

                # target corners, box0 only (x0,y0 / w0,h0 rows contiguous)
                txy0 = Bt[:, 0:2 * n]
                twh0 = Bt[:, 4 * n:6 * n]
                t2 = tmp.tile([P, 2, n], F16, tag="t2")
                nc.scalar.activation(t2, twh0, ACT.Copy, scale=3.5)
                nlt2 = tmp.tile([P, 2, n], F16, tag="nlt2")
                rt2 = tmp.tile([P, 2, n], F16, tag="rt2")
                nc.vector.tensor_tensor(nlt2, t2, txy0, op=ALU.subtract)
                nc.vector.tensor_tensor(rt2, t2, txy0, op=ALU.add)
                # rows (x,y,x,y): [boxdup step0][coord-row step n][cell step1]
                nlt2b = _mk(nlt2[:, 0, 0], [(0, 2), (n, 2), (1, n)])
                rt2b = _mk(rt2[:, 0, 0], [(0, 2), (n, 2), (1, n)])

                mln4 = tmp.tile([P, 4, n], F16, tag="mln4")
                mr4 = tmp.tile([P, 4, n], F16, tag="mr4")
                nc.vector.tensor_tensor(mln4, nl4, nlt2b, op=ALU.min)
                nc.vector.tensor_tensor(mr4, r4, rt2b, op=ALU.min)
                s4 = nl4  # dead, reuse
                nc.vector.tensor_tensor(s4, mln4, mr4, op=ALU.add)   # 7*(minr-maxl)
                cw4 = r4  # dead, reuse
                nc.scalar.activation(cw4, s4, ACT.Relu, scale=1.0 / 7.0)

                # class block here: ~8.5us of independent DVE work overlapping
                # the ACT cw4 (mid-chunk, so its DMAs are long since landed —
                # unlike class-first-at-chunk-top, which starved the head)
                nc.sync.dma_start(out=Pcl, in_=pcl_v[:, k])
                nc.sync.dma_start(out=Tcl, in_=tcl_v[:, k])
                class_block()

                # per-box scalars, box-major [P,2,n]
                inter2 = tmp.tile([P, 2, n], F16, tag="inter2")
                areap2 = tmp.tile([P, 2, n], F16, tag="areap2")
                areat = tmp.tile([P, n], F16, tag="areat")
                cwx = cw4[:, 0:4:2, :]                           # x rows {0,2}
                cwy = cw4[:, 1:4:2, :]                           # y rows {1,3}
                nc.vector.tensor_tensor(inter2, cwx, cwy, op=ALU.mult)
                pw2 = _mk(Bp[:, 4 * n], [(2 * n, 2), (1, n)])    # w rows {0,2}
                ph2 = _mk(Bp[:, 5 * n], [(2 * n, 2), (1, n)])    # h rows {1,3}
                nc.vector.tensor_tensor(areap2, pw2, ph2, op=ALU.mult)
                nc.vector.tensor_tensor(areat, Bt[:, 4 * n:5 * n], Bt[:, 5 * n:6 * n], op=ALU.mult)

                u2h = tmp.tile([P, 2, n], F16, tag="u2h")
                u2 = tmp.tile([P, 2, n], F16, tag="u2")
                nc.vector.tensor_tensor(u2h, areap2, inter2, op=ALU.subtract)
                areatb = _ins(areat[:, :], 1, 0, 2)          # [box step0][cell step1]
                nc.vector.tensor_tensor(u2, u2h, areatb, op=ALU.add)

                # call the approx-reciprocal custom op directly with fp16
                # operands: the DVE converts fp16->fp32 at read BEFORE the
                # BITWISE_NOT seed, so the fp32-bit-layout trick still holds;
                # this keeps u2 a 2x fp16 add and drops the ACT downcast hop
                from concourse.dve_ops import (
                    RECIP_APPROX_FAST_CONSTS as _RC,
                    RECIPROCAL_APPROX_FAST as _RF,
                )
                rcp16 = tmp.tile([P, 2, n], F16, tag="rcp16")
                nc.vector._custom_dve(
                    _RF, out=rcp16, in0=u2,
                    s0=_RC["s0"], s1=_RC["s1"], imm2=_RC["imm2"],
                )
                iou2 = tmp.tile([P, 2, n], F16, tag="iou2")
                nc.vector.tensor_tensor(iou2, inter2, rcp16, op=ALU.mult)

                is1 = tmp.tile([P, n], F16, tag="is1")
                riou = tmp.tile([P, n], F16, tag="riou")
                nc.vector.tensor_tensor(is1, iou2[:, 1, :], iou2[:, 0, :], op=ALU.is_gt)
                nc.vector.tensor_tensor(riou, iou2[:, 1, :], iou2[:, 0, :], op=ALU.max)

                resp = tmp.tile([P, 2, n], F16, tag="resp")
                nc.vector.tensor_tensor(resp[:, 1, :], obj_src, is1, op=ALU.mult)
                nc.vector.tensor_tensor(resp[:, 0, :], obj_src, resp[:, 1, :], op=ALU.subtract)

                # ---- losses: premask (DVE) + in-place Square-with-accum (ACT)
                # xy and wh diffs share one [P,8,n] tile; a single broadcast
                # premask multiply covers both (broadcast dims do not break
                # the 2x packed mode), then two sqaccs split the accum columns
                d8 = tmp.tile([P, 8, n], F16, tag="d8")
                dm8 = tmp.tile([P, 8, n], F16, tag="dm8")
                nc.vector.tensor_tensor(d8[:, 0:4, :], Txy, Pxy, op=ALU.subtract)
                sp4 = tmp.tile([P, 4, n], F16, tag="sp4")
                st4 = tmp.tile([P, 4, n], F16, tag="st4")
                nc.scalar.activation(sp4, Pwh, ACT.Sqrt)
                nc.scalar.activation(st4, Twh, ACT.Sqrt)
                nc.vector.tensor_tensor(d8[:, 4:8, :], st4, sp4, op=ALU.subtract)
                # rows (r0,r0,r1,r1): [box n][coorddup 0][cell 1] — 3 free dims
                # (4 exceeds the ISA AP limit); broadcast does not break 2x
                resp4b = _mk(resp[:, 0, 0], [(n, 2), (0, 2), (1, n)])
                nc.vector.tensor_tensor(dm8[:, 0:4, :], d8[:, 0:4, :], resp4b, op=ALU.mult)
                nc.vector.tensor_tensor(dm8[:, 4:8, :], d8[:, 4:8, :], resp4b, op=ALU.mult)
                sqacc(dm8[:, 0:4, :], 0)
                sqacc(dm8[:, 4:8, :], 1)

                # obj conf vs responsible-iou, box-major [P,2,n]: conf rows
                # are compact so diff and premask both pack
                dc2 = tmp.tile([P, 2, n], F16, tag="dc2")
                dmc2 = tmp.tile([P, 2, n], F16, tag="dmc2")
                rioub = _ins(riou[:, :], 1, 0, 2)                 # [boxdup][cell]
                nc.vector.tensor_tensor(dc2, rioub, Pcf, op=ALU.subtract)
                nc.vector.tensor_tensor(dmc2, dc2, resp, op=ALU.mult)
                sqacc(dmc2, 2)

                # noobj conf: noobj*(tc-pc)^2 == ((tc*pc)-pc)^2 since tc in {0,1}
                m2 = tmp.tile([P, 2, n], F16, tag="m2")
                dmn2 = tmp.tile([P, 2, n], F16, tag="dmn2")
                nc.vector.tensor_tensor(m2, Tcf, Pcf, op=ALU.mult)
                nc.vector.tensor_tensor(dmn2, m2, Pcf, op=ALU.subtract)
                sqacc(dmn2, 3)


            nc.sync.dma_start(out=out.ap(), in_=acc)

    nc.compile()
    return nc


_nc_cache = None
LAST_EXEC_NS = None
LAST_RESULT = None


def _get_nc():
    global _nc_cache
    if _nc_cache is None:
        _nc_cache = build_program()
    return _nc_cache


def _prep(full):
    """[N*S*S, 30] f32 -> per-core fp16 (box blocks [k][xy4|wh4|cf2], cls)."""
    A = np.asarray(full, dtype=np.float32).reshape(N_CORES, P, N_CH, NCK, D)
    A16 = A.astype(np.float16)
    # box-major rows: per chunk [x0,y0,x1,y1 | w0,h0,w1,h1 | c0,c1], each row
    # a contiguous n-vector
    xy = A16[..., PERM_XY].transpose(0, 1, 2, 4, 3)
    wh = A16[..., PERM_WH].transpose(0, 1, 2, 4, 3)
    cf = A16[..., PERM_CF].transpose(0, 1, 2, 4, 3)
    box = np.ascontiguousarray(np.concatenate([xy, wh, cf], axis=-2)).reshape(
        N_CORES, P, -1
    )
    cl = np.ascontiguousarray(A16[..., 10:30].transpose(0, 1, 2, 4, 3)).reshape(
        N_CORES, P, -1
    )
    return box, cl


def kernel(pred_tensor, target_tensor):
    global LAST_EXEC_NS, LAST_RESULT
    pred = np.asarray(pred_tensor).reshape(N_FULL * S * S, D)
    tgt = np.asarray(target_tensor).reshape(N_FULL * S * S, D)

    pb, pc = _prep(pred)
    tb, tc = _prep(tgt)

    in_maps = []
    for i in range(N_CORES):
        in_maps.append({"pbox": pb[i], "tbox": tb[i], "pcl": pc[i], "tcl": tc[i]})

    nc = _get_nc()
    trace = bool(os.environ.get("KERNEL_TRACE"))
    tmpdir = os.environ.get("KERNEL_TRACE_DIR") or None
    res = bass_utils.run_bass_kernel_spmd(
        nc, in_maps, core_ids=list(range(N_CORES)), trace=trace, tmpdir=tmpdir
    )
    LAST_RESULT = res
    if res.exec_time_ns is not None:
        LAST_EXEC_NS = res.exec_time_ns
    total = np.zeros(5, dtype=np.float64)
    for m in res.results:
        total += m["out"].astype(np.float64).sum(axis=0).reshape(N_CH, 5).sum(axis=0)
    losses = (total / float(N_FULL)).astype(np.float32)
    return losses


# revision 27
# speedup vs baseline: 1.0046x; 1.0046x over previous
"""YOLO-style loss kernel for Trainium2, SPMD over 8 NeuronCores.

Inputs (full): pred_tensor [32768,7,7,30] f32, target_tensor [32768,7,7,30] f32.
Output: np.ndarray shape (5,) f32 = (loss_xy, loss_wh, loss_obj, loss_noobj, loss_class).

Strategy: pure data parallel on batch dim; each core gets 4096 samples
(200704 cells). Host converts to fp16 and splits channels into fully
contiguous groups so the hot DVE ops coalesce into the 2x packed mode
(strided views of an interleaved [n,10] tile measure 1x or worse):
  - xy4  [n,4] cell-major (x0,y0,x1,y1)        both tensors
  - wh4  [n,4] cell-major (w0,h0,w1,h1)        both tensors
  - cf2  [n,2] cell-major (c0,c1)              both tensors
  - cls  [20,n] channel-major per chunk        both tensors
Per 392-cell chunk: IoU responsibility + five masked squared-diff partial
sums, fused on-chip. Weighted reductions run as premask-multiply (masks are
exactly 0/1) + in-place Square with accum_out on the scalar engine; the
class premask ANDs int32-reinterpreted fp16 pairs against a 0xFFFF mask;
reciprocal via the ~1cpe approx custom-DVE op. Each core returns a [128,20]
f32 partial-sum tile (5 losses x 4 chunks); host reduces and divides by N.
"""

import os
import sys

sys.path.insert(0, "/opt/trn_rl_repo")

import numpy as np

import concourse.bass as bass
import concourse.bacc as bacc
import concourse.tile as tile
from concourse import mybir
from concourse import bass_utils

F32 = mybir.dt.float32
F16 = mybir.dt.float16
I16 = mybir.dt.int16
I32 = mybir.dt.int32
ALU = mybir.AluOpType
ACT = mybir.ActivationFunctionType

S = 7
B = 2
C = 20
D = 30
N_FULL = 32768
N_CORES = 8
N_SHARD = N_FULL // N_CORES            # 4096 samples per core
R = N_SHARD * S * S                    # 200704 cells per core
P = 128                                # partitions
RP = R // P                            # 1568 cells per partition
NCK = 392                              # cells per partition per chunk
N_CH = RP // NCK                       # 4 chunks

PERM_XY = [0, 1, 5, 6]   # x0,y0,x1,y1
PERM_WH = [2, 3, 7, 8]   # w0,h0,w1,h1
PERM_CF = [4, 9]         # c0,c1


def _mk(ap, dims):
    """Rebuild the free dims of `ap` (keeping partition dim + offset) as
    `dims` = list of (step, count)."""
    new = [list(ap.ap[0])] + [[s, c] for s, c in dims]
    return bass.AP(tensor=ap.tensor, offset=ap.offset, ap=new)


def _ins(ap, pos, step, count):
    new = [list(x) for x in ap.ap]
    new.insert(pos, [step, count])
    return bass.AP(tensor=ap.tensor, offset=ap.offset, ap=new)


def build_program():
    nc = bacc.Bacc("TRN2", target_bir_lowering=False, debug=False)
    n = NCK

    def din(name, per_chunk):
        return nc.dram_tensor(name, [P, N_CH * per_chunk], F16, kind="ExternalInput")

    pbox, tbox = din("pbox", n * 10), din("tbox", n * 10)
    pcl, tcl = din("pcl", C * n), din("tcl", C * n)
    out = nc.dram_tensor("out", [P, 5 * N_CH], F32, kind="ExternalOutput")

    pbox_v = pbox.ap().rearrange("p (k a) -> p k a", k=N_CH, a=n * 10)
    tbox_v = tbox.ap().rearrange("p (k a) -> p k a", k=N_CH, a=n * 10)
    pcl_v = pcl.ap().rearrange("p (k c i) -> p k c i", k=N_CH, c=C, i=n)
    tcl_v = tcl.ap().rearrange("p (k c i) -> p k c i", k=N_CH, c=C, i=n)

    with tile.TileContext(nc) as tc:
        with (
            tc.tile_pool(name="raw", bufs=2) as raw,
            tc.tile_pool(name="tmp", bufs=1) as tmp,
            tc.tile_pool(name="persist", bufs=1) as persist,
        ):
            acc = persist.tile([P, 5 * N_CH], F32)

            for k in range(N_CH):
                # one block-major box DMA per tensor: [xy4(4n) | wh4(4n) | cf2(2n)]
                # contiguous inside the transfer, so every group view coalesces
                Bp = raw.tile([P, 10 * n], F16, tag="Bp")
                Bt = raw.tile([P, 10 * n], F16, tag="Bt")
                Pcl = raw.tile([P, C, n], F16, tag="Pcl")
                Tcl = raw.tile([P, C, n], F16, tag="Tcl")
                nc.sync.dma_start(out=Bp, in_=pbox_v[:, k])
                nc.sync.dma_start(out=Bt, in_=tbox_v[:, k])
                nc.sync.dma_start(out=Pcl, in_=pcl_v[:, k])
                nc.sync.dma_start(out=Tcl, in_=tcl_v[:, k])

                # box-major rows: x0,y0,x1,y1 | w0,h0,w1,h1 | c0,c1, each a
                # contiguous n-row, so every group op fully coalesces
                Pxy = Bp[:, 0:4 * n]
                Pwh = Bp[:, 4 * n:8 * n]
                Pcf = Bp[:, 8 * n:10 * n]
                Txy = Bt[:, 0:4 * n]
                Twh = Bt[:, 4 * n:8 * n]
                Tcf = Bt[:, 8 * n:10 * n]
                obj_src = Bt[:, 8 * n:9 * n]    # target c0 row, compact [P,n]

                def sqacc(dm, col):
                    # in-place square: ACT streams read-then-write per element,
                    # so out == in is safe and avoids junk tiles whose reuse
                    # would couple engines across chunks
                    nc.scalar.activation(
                        dm, dm, ACT.Square,
                        accum_out=acc[:, 5 * k + col:5 * k + col + 1],
                    )

                def class_block():
                    # class (channel-major [P,20,n]): AND the fp16 diffs
                    # against a 0xFFFF/0x0000 obj mask through int32 views —
                    # pair-packing halves the cost vs a 1x broadcast multiply
                    ffi = tmp.tile([P, n], I16, tag="ffi")
                    nc.scalar.activation(ffi, obj_src, ACT.Copy, scale=-1.0)
                    ff32 = ffi.bitcast(I32)                       # [P, n/2]
                    ff32b = _mk(ff32[:, 0], [(0, C), (1, n // 2)])
                    dcl = tmp.tile([P, C, n], F16, tag="dcl")
                    dmcl = tmp.tile([P, C, n], F16, tag="dmcl")
                    nc.vector.tensor_tensor(dcl, Tcl, Pcl, op=ALU.subtract)
                    nc.vector.tensor_tensor(
                        dmcl.bitcast(I32), dcl.bitcast(I32), ff32b,
                        op=ALU.bitwise_and,
                    )
                    sqacc(dmcl, 4)

                # ---- IoU stage (coords scaled x7: corners 3.5*wh -+ xy) ----
                # single-input scale/clamp ops ride the scalar engine (slack)
                t1 = tmp.tile([P, 4, n], F16, tag="t1")
                nc.scalar.activation(t1, Pwh, ACT.Copy, scale=3.5)
                nl4 = tmp.tile([P, 4, n], F16, tag="nl4")    # -(7l) both boxes
                r4 = tmp.tile([P, 4, n], F16, tag="r4")      # 7r both boxes
                nc.vector.tensor_tensor(nl4, t1, Pxy, op=ALU.subtract)
                nc.vector.tensor_tensor(r4, t1, Pxy, op=ALU.add)

                # target corners, box0 only (x0,y0 / w0,h0 rows contiguous)
                txy0 = Bt[:, 0:2 * n]
                twh0 = Bt[:, 4 * n:6 * n]
                t2 = tmp.tile([P, 2, n], F16, tag="t2")
                nc.scalar.activation(t2, twh0, ACT.Copy, scale=3.5)
                nlt2 = tmp.tile([P, 2, n], F16, tag="nlt2")
                rt2 = tmp.tile([P, 2, n], F16, tag="rt2")
                nc.vector.tensor_tensor(nlt2, t2, txy0, op=ALU.subtract)
                nc.vector.tensor_tensor(rt2, t2, txy0, op=ALU.add)
                # rows (x,y,x,y): [boxdup step0][coord-row step n][cell step1]
                nlt2b = _mk(nlt2[:, 0, 0], [(0, 2), (n, 2), (1, n)])
                rt2b = _mk(rt2[:, 0, 0], [(0, 2), (n, 2), (1, n)])

                mln4 = tmp.tile([P, 4, n], F16, tag="mln4")
                mr4 = tmp.tile([P, 4, n], F16, tag="mr4")
                nc.vector.tensor_tensor(mln4, nl4, nlt2b, op=ALU.min)
                nc.vector.tensor_tensor(mr4, r4, rt2b, op=ALU.min)
                s4 = nl4  # dead, reuse
                nc.vector.tensor_tensor(s4, mln4, mr4, op=ALU.add)   # 7*(minr-maxl)
                cw4 = r4  # dead, reuse
                nc.scalar.activation(cw4, s4, ACT.Relu, scale=1.0 / 7.0)

                # class block here: ~8.5us of independent DVE work overlapping
                # the ACT cw4 (mid-chunk, so its DMAs are long since landed —
                # unlike class-first-at-chunk-top, which starved the head)
                class_block()

                # per-box scalars, box-major [P,2,n]
                inter2 = tmp.tile([P, 2, n], F16, tag="inter2")
                areap2 = tmp.tile([P, 2, n], F16, tag="areap2")
                areat = tmp.tile([P, n], F16, tag="areat")
                cwx = cw4[:, 0:4:2, :]                           # x rows {0,2}
                cwy = cw4[:, 1:4:2, :]                           # y rows {1,3}
                nc.vector.tensor_tensor(inter2, cwx, cwy, op=ALU.mult)
                pw2 = _mk(Bp[:, 4 * n], [(2 * n, 2), (1, n)])    # w rows {0,2}
                ph2 = _mk(Bp[:, 5 * n], [(2 * n, 2), (1, n)])    # h rows {1,3}
                nc.vector.tensor_tensor(areap2, pw2, ph2, op=ALU.mult)
                nc.vector.tensor_tensor(areat, Bt[:, 4 * n:5 * n], Bt[:, 5 * n:6 * n], op=ALU.mult)

                u2h = tmp.tile([P, 2, n], F16, tag="u2h")
                u2 = tmp.tile([P, 2, n], F16, tag="u2")
                nc.vector.tensor_tensor(u2h, areap2, inter2, op=ALU.subtract)
                areatb = _ins(areat[:, :], 1, 0, 2)          # [box step0][cell step1]
                nc.vector.tensor_tensor(u2, u2h, areatb, op=ALU.add)

                # call the approx-reciprocal custom op directly with fp16
                # operands: the DVE converts fp16->fp32 at read BEFORE the
                # BITWISE_NOT seed, so the fp32-bit-layout trick still holds;
                # this keeps u2 a 2x fp16 add and drops the ACT downcast hop
                from concourse.dve_ops import (
                    RECIP_APPROX_FAST_CONSTS as _RC,
                    RECIPROCAL_APPROX_FAST as _RF,
                )
                rcp16 = tmp.tile([P, 2, n], F16, tag="rcp16")
                nc.vector._custom_dve(
                    _RF, out=rcp16, in0=u2,
                    s0=_RC["s0"], s1=_RC["s1"], imm2=_RC["imm2"],
                )
                iou2 = tmp.tile([P, 2, n], F16, tag="iou2")
                nc.vector.tensor_tensor(iou2, inter2, rcp16, op=ALU.mult)

                is1 = tmp.tile([P, n], F16, tag="is1")
                riou = tmp.tile([P, n], F16, tag="riou")
                nc.vector.tensor_tensor(is1, iou2[:, 1, :], iou2[:, 0, :], op=ALU.is_gt)
                nc.vector.tensor_tensor(riou, iou2[:, 1, :], iou2[:, 0, :], op=ALU.max)

                resp = tmp.tile([P, 2, n], F16, tag="resp")
                nc.vector.tensor_tensor(resp[:, 1, :], obj_src, is1, op=ALU.mult)
                nc.vector.tensor_tensor(resp[:, 0, :], obj_src, resp[:, 1, :], op=ALU.subtract)

                # ---- losses: premask (DVE) + in-place Square-with-accum (ACT)
                # xy and wh diffs share one [P,8,n] tile; a single broadcast
                # premask multiply covers both (broadcast dims do not break
                # the 2x packed mode), then two sqaccs split the accum columns
                d8 = tmp.tile([P, 8, n], F16, tag="d8")
                dm8 = tmp.tile([P, 8, n], F16, tag="dm8")
                nc.vector.tensor_tensor(d8[:, 0:4, :], Txy, Pxy, op=ALU.subtract)
                sp4 = tmp.tile([P, 4, n], F16, tag="sp4")
                st4 = tmp.tile([P, 4, n], F16, tag="st4")
                nc.scalar.activation(sp4, Pwh, ACT.Sqrt)
                nc.scalar.activation(st4, Twh, ACT.Sqrt)
                nc.vector.tensor_tensor(d8[:, 4:8, :], st4, sp4, op=ALU.subtract)
                # rows (r0,r0,r1,r1): [box n][coorddup 0][cell 1] — 3 free dims
                # (4 exceeds the ISA AP limit); broadcast does not break 2x
                resp4b = _mk(resp[:, 0, 0], [(n, 2), (0, 2), (1, n)])
                nc.vector.tensor_tensor(dm8[:, 0:4, :], d8[:, 0:4, :], resp4b, op=ALU.mult)
                nc.vector.tensor_tensor(dm8[:, 4:8, :], d8[:, 4:8, :], resp4b, op=ALU.mult)
                sqacc(dm8[:, 0:4, :], 0)
                sqacc(dm8[:, 4:8, :], 1)

                # obj conf vs responsible-iou, box-major [P,2,n]: conf rows
                # are compact so diff and premask both pack
                dc2 = tmp.tile([P, 2, n], F16, tag="dc2")
                dmc2 = tmp.tile([P, 2, n], F16, tag="dmc2")
                rioub = _ins(riou[:, :], 1, 0, 2)                 # [boxdup][cell]
                nc.vector.tensor_tensor(dc2, rioub, Pcf, op=ALU.subtract)
                nc.vector.tensor_tensor(dmc2, dc2, resp, op=ALU.mult)
                sqacc(dmc2, 2)

                # noobj conf: noobj*(tc-pc)^2 == ((tc*pc)-pc)^2 since tc in {0,1}
                m2 = tmp.tile([P, 2, n], F16, tag="m2")
                dmn2 = tmp.tile([P, 2, n], F16, tag="dmn2")
                nc.vector.tensor_tensor(m2, Tcf, Pcf, op=ALU.mult)
                nc.vector.tensor_tensor(dmn2, m2, Pcf, op=ALU.subtract)
                sqacc(dmn2, 3)


            nc.sync.dma_start(out=out.ap(), in_=acc)

    nc.compile()
    return nc


_nc_cache = None
LAST_EXEC_NS = None
LAST_RESULT = None


def _get_nc():
    global _nc_cache
    if _nc_cache is None:
        _nc_cache = build_program()
    return _nc_cache


def _prep(full):
    """[N*S*S, 30] f32 -> per-core fp16 (box blocks [k][xy4|wh4|cf2], cls)."""
    A = np.asarray(full, dtype=np.float32).reshape(N_CORES, P, N_CH, NCK, D)
    A16 = A.astype(np.float16)
    # box-major rows: per chunk [x0,y0,x1,y1 | w0,h0,w1,h1 | c0,c1], each row
    # a contiguous n-vector
    xy = A16[..., PERM_XY].transpose(0, 1, 2, 4, 3)
    wh = A16[..., PERM_WH].transpose(0, 1, 2, 4, 3)
    cf = A16[..., PERM_CF].transpose(0, 1, 2, 4, 3)
    box = np.ascontiguousarray(np.concatenate([xy, wh, cf], axis=-2)).reshape(
        N_CORES, P, -1
    )
    cl = np.ascontiguousarray(A16[..., 10:30].transpose(0, 1, 2, 4, 3)).reshape(
        N_CORES, P, -1
    )
    return box, cl


def kernel(pred_tensor, target_tensor):
    global LAST_EXEC_NS, LAST_RESULT
    pred = np.asarray(pred_tensor).reshape(N_FULL * S * S, D)
    tgt = np.asarray(target_tensor).reshape(N_FULL * S * S, D)

    pb, pc = _prep(pred)
    tb, tc = _prep(tgt)

    in_maps = []
    for i in range(N_CORES):
        in_maps.append({"pbox": pb[i], "tbox": tb[i], "pcl": pc[i], "tcl": tc[i]})

    nc = _get_nc()
    trace = bool(os.environ.get("KERNEL_TRACE"))
    tmpdir = os.environ.get("KERNEL_TRACE_DIR") or None
    res = bass_utils.run_bass_kernel_spmd(
        nc, in_maps, core_ids=list(range(N_CORES)), trace=trace, tmpdir=tmpdir
    )
    LAST_RESULT = res
    if res.exec_time_ns is not None:
        LAST_EXEC_NS = res.exec_time_ns
    total = np.zeros(5, dtype=np.float64)
    for m in res.results:
        total += m["out"].astype(np.float64).sum(axis=0).reshape(N_CH, 5).sum(axis=0)
    losses = (total / float(N_FULL)).astype(np.float32)
    return losses


# revision 28
# speedup vs baseline: 1.0082x; 1.0036x over previous
"""YOLO-style loss kernel for Trainium2, SPMD over 8 NeuronCores.

Inputs (full): pred_tensor [32768,7,7,30] f32, target_tensor [32768,7,7,30] f32.
Output: np.ndarray shape (5,) f32 = (loss_xy, loss_wh, loss_obj, loss_noobj, loss_class).

Strategy: pure data parallel on batch dim; each core gets 4096 samples
(200704 cells). Host converts to fp16 and splits channels into fully
contiguous groups so the hot DVE ops coalesce into the 2x packed mode
(strided views of an interleaved [n,10] tile measure 1x or worse):
  - xy4  [n,4] cell-major (x0,y0,x1,y1)        both tensors
  - wh4  [n,4] cell-major (w0,h0,w1,h1)        both tensors
  - cf2  [n,2] cell-major (c0,c1)              both tensors
  - cls  [20,n] channel-major per chunk        both tensors
Per 392-cell chunk: IoU responsibility + five masked squared-diff partial
sums, fused on-chip. Weighted reductions run as premask-multiply (masks are
exactly 0/1) + in-place Square with accum_out on the scalar engine; the
class premask ANDs int32-reinterpreted fp16 pairs against a 0xFFFF mask;
reciprocal via the ~1cpe approx custom-DVE op. Each core returns a [128,20]
f32 partial-sum tile (5 losses x 4 chunks); host reduces and divides by N.
"""

import os
import sys

sys.path.insert(0, "/opt/trn_rl_repo")

import numpy as np

import concourse.bass as bass
import concourse.bacc as bacc
import concourse.tile as tile
from concourse import mybir
from concourse import bass_utils

F32 = mybir.dt.float32
F16 = mybir.dt.float16
I16 = mybir.dt.int16
I32 = mybir.dt.int32
ALU = mybir.AluOpType
ACT = mybir.ActivationFunctionType

S = 7
B = 2
C = 20
D = 30
N_FULL = 32768
N_CORES = 8
N_SHARD = N_FULL // N_CORES            # 4096 samples per core
R = N_SHARD * S * S                    # 200704 cells per core
P = 128                                # partitions
RP = R // P                            # 1568 cells per partition
NCK = 392                              # cells per partition per chunk
N_CH = RP // NCK                       # 4 chunks

PERM_XY = [0, 1, 5, 6]   # x0,y0,x1,y1
PERM_WH = [2, 3, 7, 8]   # w0,h0,w1,h1
PERM_CF = [4, 9]         # c0,c1


def _mk(ap, dims):
    """Rebuild the free dims of `ap` (keeping partition dim + offset) as
    `dims` = list of (step, count)."""
    new = [list(ap.ap[0])] + [[s, c] for s, c in dims]
    return bass.AP(tensor=ap.tensor, offset=ap.offset, ap=new)


def _ins(ap, pos, step, count):
    new = [list(x) for x in ap.ap]
    new.insert(pos, [step, count])
    return bass.AP(tensor=ap.tensor, offset=ap.offset, ap=new)


def build_program():
    nc = bacc.Bacc("TRN2", target_bir_lowering=False, debug=False)
    n = NCK

    def din(name, per_chunk):
        return nc.dram_tensor(name, [P, N_CH * per_chunk], F16, kind="ExternalInput")

    pbox, tbox = din("pbox", n * 10), din("tbox", n * 10)
    pcl, tcl = din("pcl", C * n), din("tcl", C * n)
    out = nc.dram_tensor("out", [P, 5 * N_CH], F32, kind="ExternalOutput")

    pbox_v = pbox.ap().rearrange("p (k a) -> p k a", k=N_CH, a=n * 10)
    tbox_v = tbox.ap().rearrange("p (k a) -> p k a", k=N_CH, a=n * 10)
    pcl_v = pcl.ap().rearrange("p (k c i) -> p k c i", k=N_CH, c=C, i=n)
    tcl_v = tcl.ap().rearrange("p (k c i) -> p k c i", k=N_CH, c=C, i=n)

    with tile.TileContext(nc) as tc:
        with (
            tc.tile_pool(name="raw", bufs=2) as raw,
            tc.tile_pool(name="tmp", bufs=1) as tmp,
            tc.tile_pool(name="persist", bufs=1) as persist,
        ):
            acc = persist.tile([P, 5 * N_CH], F32)

            # prime the ACT table with the sqrt-anchored set before any real
            # work: every activation fn used here (Copy/Relu/Square/Sqrt) is
            # resident in it, so the second ACT_TABLE_LOAD that otherwise
            # lands mid-warmup (1.3us on chunk-0's critical path) vanishes
            warm = persist.tile([P, 1], F16)
            nc.gpsimd.memset(warm, 1.0)
            nc.scalar.activation(warm, warm, ACT.Sqrt)

            for k in range(N_CH):
                # one block-major box DMA per tensor: [xy4(4n) | wh4(4n) | cf2(2n)]
                # contiguous inside the transfer, so every group view coalesces
                Bp = raw.tile([P, 10 * n], F16, tag="Bp")
                Bt = raw.tile([P, 10 * n], F16, tag="Bt")
                Pcl = raw.tile([P, C, n], F16, tag="Pcl")
                Tcl = raw.tile([P, C, n], F16, tag="Tcl")
                nc.sync.dma_start(out=Bp, in_=pbox_v[:, k])
                nc.sync.dma_start(out=Bt, in_=tbox_v[:, k])
                nc.sync.dma_start(out=Pcl, in_=pcl_v[:, k])
                nc.sync.dma_start(out=Tcl, in_=tcl_v[:, k])

                # box-major rows: x0,y0,x1,y1 | w0,h0,w1,h1 | c0,c1, each a
                # contiguous n-row, so every group op fully coalesces
                Pxy = Bp[:, 0:4 * n]
                Pwh = Bp[:, 4 * n:8 * n]
                Pcf = Bp[:, 8 * n:10 * n]
                Txy = Bt[:, 0:4 * n]
                Twh = Bt[:, 4 * n:8 * n]
                Tcf = Bt[:, 8 * n:10 * n]
                obj_src = Bt[:, 8 * n:9 * n]    # target c0 row, compact [P,n]

                def sqacc(dm, col):
                    # in-place square: ACT streams read-then-write per element,
                    # so out == in is safe and avoids junk tiles whose reuse
                    # would couple engines across chunks
                    nc.scalar.activation(
                        dm, dm, ACT.Square,
                        accum_out=acc[:, 5 * k + col:5 * k + col + 1],
                    )

                def class_block():
                    # class (channel-major [P,20,n]): AND the fp16 diffs
                    # against a 0xFFFF/0x0000 obj mask through int32 views —
                    # pair-packing halves the cost vs a 1x broadcast multiply
                    ffi = tmp.tile([P, n], I16, tag="ffi")
                    nc.scalar.activation(ffi, obj_src, ACT.Copy, scale=-1.0)
                    ff32 = ffi.bitcast(I32)                       # [P, n/2]
                    ff32b = _mk(ff32[:, 0], [(0, C), (1, n // 2)])
                    dcl = tmp.tile([P, C, n], F16, tag="dcl")
                    dmcl = tmp.tile([P, C, n], F16, tag="dmcl")
                    nc.vector.tensor_tensor(dcl, Tcl, Pcl, op=ALU.subtract)
                    nc.vector.tensor_tensor(
                        dmcl.bitcast(I32), dcl.bitcast(I32), ff32b,
                        op=ALU.bitwise_and,
                    )
                    sqacc(dmcl, 4)

                # ---- IoU stage (coords scaled x7: corners 3.5*wh -+ xy) ----
                # single-input scale/clamp ops ride the scalar engine (slack)
                t1 = tmp.tile([P, 4, n], F16, tag="t1")
                nc.scalar.activation(t1, Pwh, ACT.Copy, scale=3.5)
                nl4 = tmp.tile([P, 4, n], F16, tag="nl4")    # -(7l) both boxes
                r4 = tmp.tile([P, 4, n], F16, tag="r4")      # 7r both boxes
                nc.vector.tensor_tensor(nl4, t1, Pxy, op=ALU.subtract)
                nc.vector.tensor_tensor(r4, t1, Pxy, op=ALU.add)

                # target corners, box0 only (x0,y0 / w0,h0 rows contiguous)
                txy0 = Bt[:, 0:2 * n]
                twh0 = Bt[:, 4 * n:6 * n]
                t2 = tmp.tile([P, 2, n], F16, tag="t2")
                nc.scalar.activation(t2, twh0, ACT.Copy, scale=3.5)
                nlt2 = tmp.tile([P, 2, n], F16, tag="nlt2")
                rt2 = tmp.tile([P, 2, n], F16, tag="rt2")
                nc.vector.tensor_tensor(nlt2, t2, txy0, op=ALU.subtract)
                nc.vector.tensor_tensor(rt2, t2, txy0, op=ALU.add)
                # rows (x,y,x,y): [boxdup step0][coord-row step n][cell step1]
                nlt2b = _mk(nlt2[:, 0, 0], [(0, 2), (n, 2), (1, n)])
                rt2b = _mk(rt2[:, 0, 0], [(0, 2), (n, 2), (1, n)])

                mln4 = tmp.tile([P, 4, n], F16, tag="mln4")
                mr4 = tmp.tile([P, 4, n], F16, tag="mr4")
                nc.vector.tensor_tensor(mln4, nl4, nlt2b, op=ALU.min)
                nc.vector.tensor_tensor(mr4, r4, rt2b, op=ALU.min)
                s4 = nl4  # dead, reuse
                nc.vector.tensor_tensor(s4, mln4, mr4, op=ALU.add)   # 7*(minr-maxl)
                cw4 = r4  # dead, reuse
                nc.scalar.activation(cw4, s4, ACT.Relu, scale=1.0 / 7.0)

                # class block here: ~8.5us of independent DVE work overlapping
                # the ACT cw4 (mid-chunk, so its DMAs are long since landed —
                # unlike class-first-at-chunk-top, which starved the head)
                class_block()

                # per-box scalars, box-major [P,2,n]
                inter2 = tmp.tile([P, 2, n], F16, tag="inter2")
                areap2 = tmp.tile([P, 2, n], F16, tag="areap2")
                areat = tmp.tile([P, n], F16, tag="areat")
                cwx = cw4[:, 0:4:2, :]                           # x rows {0,2}
                cwy = cw4[:, 1:4:2, :]                           # y rows {1,3}
                nc.vector.tensor_tensor(inter2, cwx, cwy, op=ALU.mult)
                pw2 = _mk(Bp[:, 4 * n], [(2 * n, 2), (1, n)])    # w rows {0,2}
                ph2 = _mk(Bp[:, 5 * n], [(2 * n, 2), (1, n)])    # h rows {1,3}
                nc.vector.tensor_tensor(areap2, pw2, ph2, op=ALU.mult)
                nc.vector.tensor_tensor(areat, Bt[:, 4 * n:5 * n], Bt[:, 5 * n:6 * n], op=ALU.mult)

                u2h = tmp.tile([P, 2, n], F16, tag="u2h")
                u2 = tmp.tile([P, 2, n], F16, tag="u2")
                nc.vector.tensor_tensor(u2h, areap2, inter2, op=ALU.subtract)
                areatb = _ins(areat[:, :], 1, 0, 2)          # [box step0][cell step1]
                nc.vector.tensor_tensor(u2, u2h, areatb, op=ALU.add)

                # call the approx-reciprocal custom op directly with fp16
                # operands: the DVE converts fp16->fp32 at read BEFORE the
                # BITWISE_NOT seed, so the fp32-bit-layout trick still holds;
                # this keeps u2 a 2x fp16 add and drops the ACT downcast hop
                from concourse.dve_ops import (
                    RECIP_APPROX_FAST_CONSTS as _RC,
                    RECIPROCAL_APPROX_FAST as _RF,
                )
                rcp16 = tmp.tile([P, 2, n], F16, tag="rcp16")
                nc.vector._custom_dve(
                    _RF, out=rcp16, in0=u2,
                    s0=_RC["s0"], s1=_RC["s1"], imm2=_RC["imm2"],
                )
                iou2 = tmp.tile([P, 2, n], F16, tag="iou2")
                nc.vector.tensor_tensor(iou2, inter2, rcp16, op=ALU.mult)

                is1 = tmp.tile([P, n], F16, tag="is1")
                riou = tmp.tile([P, n], F16, tag="riou")
                nc.vector.tensor_tensor(is1, iou2[:, 1, :], iou2[:, 0, :], op=ALU.is_gt)
                nc.vector.tensor_tensor(riou, iou2[:, 1, :], iou2[:, 0, :], op=ALU.max)

                resp = tmp.tile([P, 2, n], F16, tag="resp")
                nc.vector.tensor_tensor(resp[:, 1, :], obj_src, is1, op=ALU.mult)
                nc.vector.tensor_tensor(resp[:, 0, :], obj_src, resp[:, 1, :], op=ALU.subtract)

                # ---- losses: premask (DVE) + in-place Square-with-accum (ACT)
                # xy and wh diffs share one [P,8,n] tile; a single broadcast
                # premask multiply covers both (broadcast dims do not break
                # the 2x packed mode), then two sqaccs split the accum columns
                d8 = tmp.tile([P, 8, n], F16, tag="d8")
                dm8 = tmp.tile([P, 8, n], F16, tag="dm8")
                nc.vector.tensor_tensor(d8[:, 0:4, :], Txy, Pxy, op=ALU.subtract)
                sp4 = tmp.tile([P, 4, n], F16, tag="sp4")
                st4 = tmp.tile([P, 4, n], F16, tag="st4")
                nc.scalar.activation(sp4, Pwh, ACT.Sqrt)
                nc.scalar.activation(st4, Twh, ACT.Sqrt)
                nc.vector.tensor_tensor(d8[:, 4:8, :], st4, sp4, op=ALU.subtract)
                # rows (r0,r0,r1,r1): [box n][coorddup 0][cell 1] — 3 free dims
                # (4 exceeds the ISA AP limit); broadcast does not break 2x
                resp4b = _mk(resp[:, 0, 0], [(n, 2), (0, 2), (1, n)])
                nc.vector.tensor_tensor(dm8[:, 0:4, :], d8[:, 0:4, :], resp4b, op=ALU.mult)
                nc.vector.tensor_tensor(dm8[:, 4:8, :], d8[:, 4:8, :], resp4b, op=ALU.mult)
                sqacc(dm8[:, 0:4, :], 0)
                sqacc(dm8[:, 4:8, :], 1)

                # obj conf vs responsible-iou, box-major [P,2,n]: conf rows
                # are compact so diff and premask both pack
                dc2 = tmp.tile([P, 2, n], F16, tag="dc2")
                dmc2 = tmp.tile([P, 2, n], F16, tag="dmc2")
                rioub = _ins(riou[:, :], 1, 0, 2)                 # [boxdup][cell]
                nc.vector.tensor_tensor(dc2, rioub, Pcf, op=ALU.subtract)
                nc.vector.tensor_tensor(dmc2, dc2, resp, op=ALU.mult)
                sqacc(dmc2, 2)

                # noobj conf: noobj*(tc-pc)^2 == ((tc*pc)-pc)^2 since tc in {0,1}
                m2 = tmp.tile([P, 2, n], F16, tag="m2")
                dmn2 = tmp.tile([P, 2, n], F16, tag="dmn2")
                nc.vector.tensor_tensor(m2, Tcf, Pcf, op=ALU.mult)
                nc.vector.tensor_tensor(dmn2, m2, Pcf, op=ALU.subtract)
                sqacc(dmn2, 3)


            nc.sync.dma_start(out=out.ap(), in_=acc)

    nc.compile()
    return nc


_nc_cache = None
LAST_EXEC_NS = None
LAST_RESULT = None


def _get_nc():
    global _nc_cache
    if _nc_cache is None:
        _nc_cache = build_program()
    return _nc_cache


def _prep(full):
    """[N*S*S, 30] f32 -> per-core fp16 (box blocks [k][xy4|wh4|cf2], cls)."""
    A = np.asarray(full, dtype=np.float32).reshape(N_CORES, P, N_CH, NCK, D)
    A16 = A.astype(np.float16)
    # box-major rows: per chunk [x0,y0,x1,y1 | w0,h0,w1,h1 | c0,c1], each row
    # a contiguous n-vector
    xy = A16[..., PERM_XY].transpose(0, 1, 2, 4, 3)
    wh = A16[..., PERM_WH].transpose(0, 1, 2, 4, 3)
    cf = A16[..., PERM_CF].transpose(0, 1, 2, 4, 3)
    box = np.ascontiguousarray(np.concatenate([xy, wh, cf], axis=-2)).reshape(
        N_CORES, P, -1
    )
    cl = np.ascontiguousarray(A16[..., 10:30].transpose(0, 1, 2, 4, 3)).reshape(
        N_CORES, P, -1
    )
    return box, cl


def kernel(pred_tensor, target_tensor):
    global LAST_EXEC_NS, LAST_RESULT
    pred = np.asarray(pred_tensor).reshape(N_FULL * S * S, D)
    tgt = np.asarray(target_tensor).reshape(N_FULL * S * S, D)

    pb, pc = _prep(pred)
    tb, tc = _prep(tgt)

    in_maps = []
    for i in range(N_CORES):
        in_maps.append({"pbox": pb[i], "tbox": tb[i], "pcl": pc[i], "tcl": tc[i]})

    nc = _get_nc()
    trace = bool(os.environ.get("KERNEL_TRACE"))
    tmpdir = os.environ.get("KERNEL_TRACE_DIR") or None
    res = bass_utils.run_bass_kernel_spmd(
        nc, in_maps, core_ids=list(range(N_CORES)), trace=trace, tmpdir=tmpdir
    )
    LAST_RESULT = res
    if res.exec_time_ns is not None:
        LAST_EXEC_NS = res.exec_time_ns
    total = np.zeros(5, dtype=np.float64)
    for m in res.results:
        total += m["out"].astype(np.float64).sum(axis=0).reshape(N_CH, 5).sum(axis=0)
    losses = (total / float(N_FULL)).astype(np.float32)
    return losses
